# revision 31
# baseline (speedup 1.0000x reference)
"""Trainium2 Bass kernel for nn_ScoreGraphReconstructor (3-layer GATv2 + edge MLP).

Sharding: nodes are permuted into 8*WPC windows of 128 slots, balanced by
in-degree. Each core owns WPC windows (contiguous slot range) and all edges
whose *target* lands in its windows, so the segment softmax/scatter is fully
core-local.

v2 design (bf16 data path):
- All matmuls bf16 (fp8 one-hot stationaries), PSUM fp32 accumulate.
- Per layer: local node matmuls xl/xr (bf16) -> AllGather xl (bf16, HBM) ->
  edge phase per window: dma_gather of source transforms (bf16, 512B/edge),
  xr broadcast to edge slots via precomputed one-hot matmul (st2) instead of
  a second gather, leaky-relu on the scalar engine (Prelu), att-weighted
  group-reduce for logits on vector, exp on scalar, one-hot scatter matmul
  (st) accumulating [weighted features | softmax denominator] in PSUM.
- Edge classifier: transposed bf16 gathers from AllGathered h3, all-bf16
  matmuls, ELU via 2 scalar passes + 1 fused vector op, biases folded into
  scalar-engine activations or K=1 bf16 matmuls.
"""

import sys

for _p in ("/opt/trn_rl_repo",):
    if _p not in sys.path:
        sys.path.insert(0, _p)

import numpy as np
import ml_dtypes

import concourse.bass as bass
import concourse.bacc as bacc
import concourse.mybir as mybir
import concourse.tile as tile
from concourse.bass_utils import run_bass_kernel_spmd

F32 = mybir.dt.float32
BF16 = mybir.dt.bfloat16
FP8 = mybir.dt.float8e4
I16 = mybir.dt.int16

NPBF16 = ml_dtypes.bfloat16
NPFP8 = ml_dtypes.float8_e4m3

NCORES = 8
H, C = 4, 64
HID = H * C  # 256
NCLS = 5


class Cfg:
    def __init__(self, n_nodes, n_edges, wpc, T, in_dim=256):
        self.n = n_nodes
        self.e = n_edges
        self.wpc = wpc                    # windows per core
        self.T = T                        # edge tiles (of 128) per window
        self.L = wpc * 128                # local slots per core
        self.S = NCORES * self.L          # total slots
        self.nwin = NCORES * self.wpc
        self.ew = T * 128                 # edge slots per window
        self.in_dim = in_dim
        self.ec = -(-n_edges // NCORES)   # classifier edges per core (unpadded)
        self.ecp = -(-self.ec // 512) * 512  # padded to 512


# ---------------------------------------------------------------- host prep


def _balance_windows(deg, nwin):
    """Assign node n (with weight deg[n]) to one of nwin windows, each holding
    exactly 128 nodes (rest dummy), minimizing max window load. Greedy LPT."""
    import heapq

    n = len(deg)
    order = np.argsort(-deg, kind="stable")
    heap = [(0, w) for w in range(nwin)]
    heapq.heapify(heap)
    count = np.zeros(nwin, np.int64)
    slot_of = np.empty(n, np.int64)
    pos = np.zeros(nwin, np.int64)
    for node in order:
        while True:
            load, w = heapq.heappop(heap)
            if count[w] < 128:
                break
        slot_of[node] = w * 128 + pos[w]
        pos[w] += 1
        count[w] += 1
        if count[w] < 128:
            heapq.heappush(heap, (load + int(deg[node]), w))
    return slot_of


def _wrap_idx16(idx, rows=128):
    """dma_gather index layout: index i lives at [i % 16, i // 16] of a
    (rows, len/16) int16 SBUF tile; rows 16..127 padded with zeros."""
    n = len(idx)
    assert n % 16 == 0
    blk = np.asarray(idx, np.int16).reshape(n // 16, 16).T
    return np.tile(blk, (rows // 16, 1))


def prepare_host(inputs, cfg):
    """Build per-core input maps + metadata. inputs: dict from setup_inputs."""
    cn = {k: np.asarray(v) for k, v in inputs.items()}
    x = cn["x"].astype(np.float32)
    ei = cn["edge_index"].astype(np.int64)
    row, col = ei[0], ei[1]
    n, e = cfg.n, cfg.e
    L, S, wpc, T = cfg.L, cfg.S, cfg.wpc, cfg.T

    deg = np.bincount(col, minlength=n) + 1  # +1 self loop
    slot_of = _balance_windows(deg, cfg.nwin)

    # --- edge lists (conv graph: original edges + self loops on ALL slots)
    all_slots = np.arange(S, dtype=np.int64)
    src_sl = np.concatenate([slot_of[row], all_slots])
    dst_sl = np.concatenate([slot_of[col], all_slots])
    win = dst_sl // 128
    ordr = np.argsort(win, kind="stable")
    src_sl, dst_sl, win = src_sl[ordr], dst_sl[ordr], win[ordr]
    starts = np.searchsorted(win, np.arange(cfg.nwin))
    ends = np.searchsorted(win, np.arange(cfg.nwin), side="right")
    maxcnt = int((ends - starts).max())
    assert maxcnt <= cfg.ew, f"window overflow: {maxcnt} > {cfg.ew}; raise T"

    # per-core tensors: gather indices + one-hot scatter/broadcast matrices
    per_core = []
    for c in range(NCORES):
        esrc = np.zeros((128, wpc, cfg.ew // 16), np.int16)
        st = np.zeros((128, wpc, T, 128), NPFP8)   # [edge_part, dst] scatter
        st2 = np.zeros((128, wpc, T, 128), NPFP8)  # [dst_part, edge] broadcast
        for wi in range(wpc):
            w = c * wpc + wi
            s0, s1 = starts[w], ends[w]
            cnt = s1 - s0
            srcw = np.zeros(cfg.ew, np.int64)
            dofw = np.full(cfg.ew, 128, np.int64)
            srcw[:cnt] = src_sl[s0:s1]
            dofw[:cnt] = dst_sl[s0:s1] - w * 128
            esrc[:, wi, :] = _wrap_idx16(srcw)
            # edge ordinal i -> partition i%128, tile i//128 (gather layout)
            dof2 = dofw.reshape(T, 128)  # [t, p]
            for t in range(T):
                valid = dof2[t] < 128
                p_idx = np.nonzero(valid)[0]
                d_idx = dof2[t][valid]
                st[p_idx, wi, t, d_idx] = 1.0
                st2[d_idx, wi, t, p_idx] = 1.0
        per_core.append({
            "esrc": esrc,
            "st": st.reshape(128, wpc * T * 128),
            "st2": st2.reshape(128, wpc * T * 128),
        })

    # --- node features, permuted + transposed (bf16)
    xp = np.zeros((S, cfg.in_dim), np.float32)
    xp[slot_of] = x
    nin = cfg.in_dim // 128

    # --- weights (bf16) with ELU(-1) folds
    wmaps = {}
    flags = {}
    for l in (1, 2, 3):
        Wl = cn[f"W{l}l"].astype(np.float32)
        Wr = cn[f"W{l}r"].astype(np.float32)
        bl = cn[f"b{l}l"].astype(np.float32).copy()
        br = cn[f"b{l}r"].astype(np.float32).copy()
        att = cn[f"att{l}"].astype(np.float32)
        wmaps[f"Wl{l}"] = Wl.astype(NPBF16)
        wmaps[f"Wr{l}"] = Wr.astype(NPBF16)
        wmaps[f"bl{l}"] = bl.reshape(1, HID).astype(NPBF16)
        wmaps[f"br{l}"] = br.reshape(1, HID).astype(NPBF16)
        flags[f"bl{l}"] = bool(np.any(bl != 0))
        flags[f"br{l}"] = bool(np.any(br != 0))
        wmaps[f"attb{l}"] = np.tile(att.reshape(1, HID), (128, 1)).astype(NPBF16)
        gb = cn[f"bias{l}"].astype(np.float32)
        if l < 3:
            wmaps[f"biasb{l}"] = np.tile(gb.reshape(1, HID), (128, 1)).astype(NPBF16)
            flags[f"biasb{l}"] = bool(np.any(gb != 0))
        else:
            bias3 = gb  # folded into classifier bc1 below

    Wc1 = cn["Wc1"].astype(np.float32)
    Wc2 = cn["Wc2"].astype(np.float32)
    Wc3 = cn["Wc3"].astype(np.float32)
    bc1 = cn["bc1"].astype(np.float32) + np.concatenate([bias3, bias3]) @ Wc1
    bc2 = cn["bc2"].astype(np.float32)
    bc3 = cn["bc3"].astype(np.float32)
    wmaps["Wc1"] = Wc1.astype(NPBF16)
    wmaps["Wc2"] = Wc2.astype(NPBF16)
    wmaps["Wc3"] = Wc3.astype(NPBF16)
    wmaps["bc1"] = bc1.reshape(1, HID).astype(NPBF16)
    flags["bc1"] = bool(np.any(bc1 != 0))
    wmaps["bc2"] = bc2.reshape(1, HID // 2).astype(NPBF16)
    flags["bc2"] = bool(np.any(bc2 != 0))
    wmaps["bc3c"] = bc3.reshape(NCLS, 1).astype(np.float32)
    wmaps["ones512"] = np.ones((1, 512), NPBF16)
    wmaps["ident"] = np.eye(128, dtype=NPBF16)
    wmaps["ident8"] = np.eye(128, dtype=NPFP8)

    # --- classifier edge shards (original edge order)
    in_maps = []
    for c in range(NCORES):
        m = dict(wmaps)
        m.update(per_core[c])
        xc = xp[c * L : (c + 1) * L].T.copy()  # (in_dim, L)
        m["xfm"] = xc.reshape(nin, 128, L).astype(NPBF16)
        e0 = min(c * cfg.ec, e)
        e1 = min(e0 + cfg.ec, e)
        rowp = np.zeros(cfg.ecp, np.int64)
        colp = np.zeros(cfg.ecp, np.int64)
        rowp[: e1 - e0] = slot_of[row[e0:e1]]
        colp[: e1 - e0] = slot_of[col[e0:e1]]
        m["crow"] = _wrap_idx16(rowp)
        m["ccol"] = _wrap_idx16(colp)
        in_maps.append(m)

    meta = {"slot_of": slot_of, "cfg": cfg, "flags": flags}
    return in_maps, meta


# ---------------------------------------------------------------- device build


def build_nc(cfg, flags, debug=False):
    nc = bacc.Bacc(
        "TRN2",
        target_bir_lowering=False,
        debug=debug,
        num_devices=NCORES,
        num_swdge_queues=1,
    )
    L, S, wpc, T, ew = cfg.L, cfg.S, cfg.wpc, cfg.T, cfg.ew
    nin = cfg.in_dim // 128
    ecp = cfg.ecp
    AF = mybir.ActivationFunctionType
    OP = mybir.AluOpType

    # ---- I/O declarations
    P = {}

    def pin(name, shape, dtype=BF16):
        P[name] = nc.declare_dram_parameter(name, list(shape), dtype, isOutput=False)

    pin("xfm", (nin, 128, L))
    pin("esrc", (128, wpc, ew // 16), I16)
    pin("st", (128, wpc * T * 128), FP8)
    pin("st2", (128, wpc * T * 128), FP8)
    for l in (1, 2, 3):
        pin(f"Wl{l}", (cfg.in_dim if l == 1 else HID, HID))
        pin(f"Wr{l}", (cfg.in_dim if l == 1 else HID, HID))
        if flags[f"bl{l}"]:
            pin(f"bl{l}", (1, HID))
        if flags[f"br{l}"]:
            pin(f"br{l}", (1, HID))
        pin(f"attb{l}", (128, HID))
        if l < 3 and flags[f"biasb{l}"]:
            pin(f"biasb{l}", (128, HID))
    pin("Wc1", (2 * HID, HID))
    pin("Wc2", (HID, HID // 2))
    pin("Wc3", (HID // 2, NCLS))
    if flags["bc1"]:
        pin("bc1", (1, HID))
    if flags["bc2"]:
        pin("bc2", (1, HID // 2))
    pin("bc3c", (NCLS, 1), F32)
    pin("ones512", (1, 512))
    pin("ident", (128, 128))
    pin("ident8", (128, 128), FP8)
    pin("crow", (128, ecp // 16), I16)
    pin("ccol", (128, ecp // 16), I16)
    out_t = nc.declare_dram_parameter("out_t", [NCLS, ecp], F32, isOutput=True)

    rg = [list(range(NCORES))]
    # rt psum processed in groups of TG tiles (PSUM is 8 banks of 2KB)
    TG = 2

    with tile.TileContext(nc) as tc:
        with (
            tc.tile_pool(name="const", bufs=1) as cp,
            tc.tile_pool(name="dram", bufs=1, space="DRAM") as dp,
            tc.tile_pool(name="work", bufs=2) as wp,
            tc.tile_pool(name="epbig", bufs=2) as ep,
            tc.tile_pool(name="psrt", bufs=4, space="PSUM") as psrt,
            tc.tile_pool(name="psout", bufs=1, space="PSUM") as psout,
            tc.tile_pool(name="pstp", bufs=1, space="PSUM") as pstp,
            tc.tile_pool(name="psz3", bufs=1, space="PSUM") as psz3,
        ):
            # ---------- constants into SBUF
            # tensors with >128 rows are stored as column-chunks:
            # rows [c*128:(c+1)*128] live at columns [c*s1:(c+1)*s1]
            def load_const(name, dtype=BF16):
                src = P[name]
                shp = list(src.shape)
                if len(shp) == 2 and shp[0] > 128:
                    s0, s1 = shp
                    nchunk = s0 // 128
                    t = cp.tile([128, nchunk * s1], dtype, tag=name, name=name + "_sb")
                    for c in range(nchunk):
                        nc.sync.dma_start(
                            t[:, c * s1 : (c + 1) * s1], src[c * 128 : (c + 1) * 128, :]
                        )
                else:
                    t = cp.tile(shp, dtype, tag=name, name=name + "_sb")
                    nc.sync.dma_start(t[:], src[:])
                return t

            def wchunk(t, name, c):
                """chunk c (rows c*128..) of a >128-row const loaded by load_const."""
                s1 = P[name].shape[1]
                return t[:, c * s1 : (c + 1) * s1]

            xfm = [
                cp.tile([128, L], BF16, tag=f"xfm{c}", name=f"xfm{c}")
                for c in range(nin)
            ]
            for c in range(nin):
                nc.sync.dma_start(xfm[c][:], P["xfm"][c])
            esrc_sb = load_const("esrc", dtype=I16)
            st_sb = load_const("st", dtype=FP8)
            st2_sb = load_const("st2", dtype=FP8)
            stv = st_sb[:].rearrange("p (w t d) -> p w t d", t=T, d=128)
            st2v = st2_sb[:].rearrange("p (w t d) -> p w t d", t=T, d=128)
            consts = {}
            for l in (1, 2, 3):
                for nm in (f"Wl{l}", f"Wr{l}", f"bl{l}", f"br{l}", f"attb{l}",
                           f"biasb{l}"):
                    if nm in P:
                        consts[nm] = load_const(nm)
            for nm in ("Wc1", "Wc2", "Wc3", "bc1", "bc2", "ones512"):
                if nm in P:
                    consts[nm] = load_const(nm)
            consts["bc3c"] = load_const("bc3c", dtype=F32)
            crow_sb = load_const("crow", dtype=I16)
            ccol_sb = load_const("ccol", dtype=I16)
            ident = load_const("ident")
            ident8 = load_const("ident8", dtype=FP8)

            hbuf = cp.tile([128, wpc, HID], BF16, tag="hbuf")
            xr_sb = cp.tile([128, wpc, HID], BF16, tag="xr_sb")

            # DRAM scratch
            xl_in = {
                l: dp.tile([L, HID], BF16, tag=f"xl_in{l}", name=f"xl_in{l}")
                for l in (1, 2, 3)
            }
            xl_full = {
                l: dp.tile([S, HID], BF16, tag=f"xl_full{l}", name=f"xl_full{l}",
                            addr_space="Shared")
                for l in (1, 2, 3)
            }
            h3_in = dp.tile([L, HID], BF16, tag="h3_in")
            h3_full = dp.tile([S, HID], BF16, tag="h3_full", addr_space="Shared")

            ones = consts["ones512"]

            # ================= layers
            for l in (1, 2, 3):
                kin = cfg.in_dim if l == 1 else HID
                nk = kin // 128
                # ---- transposes of hbuf into xfm for l >= 2
                if l >= 2:
                    for w in range(wpc):
                        ws = slice(w * 128, (w + 1) * 128)
                        for c in range(nk):
                            tps = pstp.tile([128, 128], BF16, tag="tps")
                            nc.tensor.transpose(
                                tps[:], hbuf[:, w, c * 128 : (c + 1) * 128], ident[:]
                            )
                            nc.scalar.activation(xfm[c][:, ws], tps[:], AF.Copy)
                # ---- node matmuls: xl (to DRAM for AllGather), xr (SBUF)
                for w in range(wpc):
                    for side, Wn, bn, to_dram in (
                        ("l", f"Wl{l}", f"bl{l}", True),
                        ("r", f"Wr{l}", f"br{l}", False),
                    ):
                        ws = slice(w * 128, (w + 1) * 128)
                        ps = psout.tile([128, HID + H], F32,
                                        tag=f"ops{0 if to_dram else 1}",
                                        name="ps_node")
                        pso = ps[:, :HID]
                        for c in range(nk):
                            nc.tensor.matmul(
                                pso,
                                lhsT=xfm[c][:, ws],
                                rhs=wchunk(consts[Wn], Wn, c),
                                start=(c == 0),
                                stop=(c == nk - 1 and bn not in consts),
                            )
                        if bn in consts:
                            # bias via K=1 outer product: ones(1,128)^T @ b(1,256)
                            nc.tensor.matmul(
                                pso,
                                lhsT=ones[:1, 0:128],
                                rhs=consts[bn][:1, :],
                                start=False,
                                stop=True,
                            )
                        if to_dram:
                            xo = wp.tile([128, HID], BF16, tag="xo")
                            nc.scalar.activation(xo[:], pso, AF.Copy)
                            nc.sync.dma_start(xl_in[l][ws, :], xo[:])
                        else:
                            nc.scalar.activation(xr_sb[:, w, :], pso, AF.Copy)

                # ---- AllGather xl
                nc.gpsimd.collective_compute(
                    "AllGather",
                    OP.bypass,
                    replica_groups=rg,
                    ins=[xl_in[l][:].opt()],
                    outs=[xl_full[l][:].opt()],
                )

                # ---- edge phase: windows processed in pairs (fewer, larger
                # gathers and vector ops; deeper cross-engine pipelining)
                attb = consts[f"attb{l}"]
                for w0 in range(0, wpc, 2):
                    nw = min(2, wpc - w0)
                    nt = nw * T
                    gt = ep.tile([128, 2 * T, HID], BF16, tag="gt")
                    gv = gt[:, :nt, :]
                    nc.gpsimd.dma_gather(
                        out_ap=gv,
                        in_ap=xl_full[l][:],
                        idxs_ap=esrc_sb[:, w0 : w0 + nw, :],
                        num_idxs=nw * ew,
                        num_idxs_reg=nw * ew,
                        elem_size=HID,
                        single_packet=False,
                        queue_num=0,
                    )
                    # A = st2 @ xr + ident @ gt, all accumulated on the PE;
                    # leaky via Prelu on scalar straight from PSUM.
                    am = ep.tile([128, 2 * T, HID], BF16, tag="am")
                    av = am[:, :nt, :]
                    for g0 in range(0, nt, TG):
                        gn = min(TG, nt - g0)
                        rt = psrt.tile([128, TG, HID], F32, tag="rt")
                        # spanning start=True matmul first (zeroes + writes gt),
                        # then per-tile accumulates — the reverse order (started
                        # sub-regions, then a spanning accumulate) drops the
                        # spanning matmul's contribution on HW.
                        nc.tensor.matmul(
                            rt[:, :gn, :],
                            lhsT=ident8[:],
                            rhs=gt[:, g0 : g0 + gn, :],
                            start=True,
                            stop=False,
                        )
                        for t in range(gn):
                            gt_t = g0 + t
                            nc.tensor.matmul(
                                rt[:, t, :],
                                lhsT=st2v[:, w0 + gt_t // T, gt_t % T, :],
                                rhs=xr_sb[:, w0 + gt_t // T, :],
                                start=False,
                                stop=(t == gn - 1),
                            )
                        nc.scalar.activation(
                            am[:, g0 : g0 + gn, :], rt[:, :gn, :],
                            AF.Prelu, alpha=0.2,
                        )
                    # F = E*att
                    nc.vector.tensor_tensor(
                        out=av,
                        in0=av,
                        in1=attb[:].unsqueeze(1).to_broadcast([128, nt, HID]),
                        op=OP.mult,
                    )
                    lg = ep.tile([128, 2 * T, H], F32, tag="lg")
                    nc.vector.tensor_reduce(
                        out=lg[:, :nt, :],
                        in_=av.rearrange("p t (h c) -> p t h c", c=C),
                        axis=mybir.AxisListType.X,
                        op=OP.add,
                    )
                    wx = ep.tile([128, 2 * T, HID + H], BF16, tag="wx")
                    nc.scalar.activation(
                        wx[:, :nt, HID : HID + H], lg[:, :nt, :], AF.Exp
                    )
                    nc.vector.tensor_tensor(
                        out=wx[:, :nt, 0:HID].rearrange("p t (h c) -> p t h c", c=C),
                        in0=gv.rearrange("p t (h c) -> p t h c", c=C),
                        in1=wx[:, :nt, HID : HID + H]
                        .unsqueeze(3)
                        .to_broadcast([128, nt, H, C]),
                        op=OP.mult,
                    )
                    opsl = [
                        psout.tile([128, HID + H], F32, tag=f"ops{wi}",
                                   name=f"ops{wi}")
                        for wi in range(nw)
                    ]
                    for wi in range(nw):
                        for t in range(T):
                            nc.tensor.matmul(
                                opsl[wi][:],
                                lhsT=stv[:, w0 + wi, t, :],
                                rhs=wx[:, wi * T + t, :],
                                start=(t == 0),
                                stop=(t == T - 1),
                            )
                    rc = ep.tile([128, 2, H], F32, tag="rc")
                    for wi in range(nw):
                        nc.vector.reciprocal(
                            rc[:, wi, :], opsl[wi][:, HID : HID + H]
                        )
                        nc.vector.tensor_tensor(
                            out=hbuf[:, w0 + wi, :].rearrange("p (h c) -> p h c", c=C),
                            in0=opsl[wi][:, 0:HID].rearrange("p (h c) -> p h c", c=C),
                            in1=rc[:, wi, :].unsqueeze(2).to_broadcast([128, H, C]),
                            op=OP.mult,
                        )

                if l < 3 and flags[f"biasb{l}"]:
                    nc.vector.tensor_tensor(
                        out=hbuf[:],
                        in0=hbuf[:],
                        in1=consts[f"biasb{l}"][:]
                        .unsqueeze(1)
                        .to_broadcast([128, wpc, HID]),
                        op=OP.add,
                    )

                if l <= 2:
                    # ELU = exp(min(h,0)) - 1 + max(h,0). Storing elu+1 in
                    # bf16 and folding -1 into the next bias loses absolute
                    # precision for small h; subtract the 1 here instead.
                    eg = 5
                    for g in range(0, wpc, eg):
                        ge = min(g + eg, wpc)
                        sl = hbuf[:, g:ge, :]
                        # te holds exp(min(h,0)) ~ 1 for small h; keep fp32 so
                        # the -1 subtraction doesn't amplify rounding error
                        te = wp.tile([128, eg, HID], F32, tag="te")
                        tp = wp.tile([128, eg, HID], BF16, tag="tp")
                        tv = te[:, : ge - g, :]
                        pv = tp[:, : ge - g, :]
                        nc.scalar.activation(tv, sl, AF.Relu, scale=-1.0)
                        nc.scalar.activation(tv, tv, AF.Exp, scale=-1.0)
                        nc.scalar.activation(pv, sl, AF.Relu)
                        nc.vector.scalar_tensor_tensor(
                            out=sl, in0=tv, scalar=-1.0, in1=pv,
                            op0=OP.add, op1=OP.add,
                        )
                else:
                    for w in range(wpc):
                        nc.sync.dma_start(
                            h3_in[w * 128 : (w + 1) * 128, :], hbuf[:, w, :]
                        )

            # ---- AllGather h3 (bf16)
            nc.gpsimd.collective_compute(
                "AllGather",
                OP.bypass,
                replica_groups=rg,
                ins=[h3_in[:].opt()],
                outs=[h3_full[:].opt()],
            )

            # ================= classifier
            Wc1b = consts["Wc1"]
            Wc2b = consts["Wc2"]
            Wc3b = consts["Wc3"]
            bc3c = consts["bc3c"]

            def elu_fused(dst, zp):
                """dst(bf16) = elu(zp) = exp(min(zp,0)) - 1 + max(zp,0).
                The -1 must be applied before any bf16 rounding of the
                exp term (~1 for small z), so te stays fp32 and the fused
                vector op does (te - 1) + relu(z). relu(z) on vector to
                offload the classifier-phase-saturated scalar engine."""
                te = ep.tile([128, 512], F32, tag="clf_te")
                tp = ep.tile([128, 512], BF16, tag="clf_tp")
                nc.scalar.activation(te[:], zp[:], AF.Relu, scale=-1.0)
                nc.scalar.activation(te[:], te[:], AF.Exp, scale=-1.0)
                nc.vector.tensor_scalar(
                    out=tp[:], in0=zp[:], scalar1=0.0, scalar2=None, op0=OP.max
                )
                nc.vector.scalar_tensor_tensor(
                    out=dst, in0=te[:], scalar=-1.0, in1=tp[:],
                    op0=OP.add, op1=OP.add,
                )

            # blocks of 512 edges, gathered in pairs (1024 idx per gather)
            for pb in range(ecp // 1024):
                ps_ = slice(pb * 64, (pb + 1) * 64)
                ut = ep.tile([128, HID // 128, 1024], BF16, tag="ut")
                nc.gpsimd.dma_gather(
                    out_ap=ut[:], in_ap=h3_full[:], idxs_ap=crow_sb[:, ps_],
                    num_idxs=1024, num_idxs_reg=1024, elem_size=HID,
                    transpose=True, single_packet=False, queue_num=0,
                )
                vt = ep.tile([128, HID // 128, 1024], BF16, tag="vt")
                nc.gpsimd.dma_gather(
                    out_ap=vt[:], in_ap=h3_full[:], idxs_ap=ccol_sb[:, ps_],
                    num_idxs=1024, num_idxs_reg=1024, elem_size=HID,
                    transpose=True, single_packet=False, queue_num=0,
                )
                for sb in range(2):
                    b = 2 * pb + sb
                    ss = slice(sb * 512, (sb + 1) * 512)
                    z1 = ep.tile([128, 2, 512], BF16, tag="z1")
                    for m in range(2):
                        ms = slice(m * 128, (m + 1) * 128)
                        zpt = psrt.tile([128, TG, HID], F32, tag="rt",
                                        name="zp1")
                        zp = zpt[:].rearrange("p a b -> p (a b)")
                        for c in range(2):
                            nc.tensor.matmul(
                                zp[:],
                                lhsT=Wc1b[
                                    :, c * HID + m * 128 : c * HID + (m + 1) * 128
                                ],
                                rhs=ut[:, c, ss], start=(c == 0), stop=False,
                            )
                        for c in range(2):
                            cc = 2 + c
                            last = (c == 1) and not flags["bc1"]
                            nc.tensor.matmul(
                                zp[:],
                                lhsT=Wc1b[
                                    :, cc * HID + m * 128 : cc * HID + (m + 1) * 128
                                ],
                                rhs=vt[:, c, ss], start=False, stop=last,
                            )
                        if flags["bc1"]:
                            nc.tensor.matmul(
                                zp[:], lhsT=consts["bc1"][:1, ms], rhs=ones[:1, :],
                                start=False, stop=True,
                            )
                        elu_fused(z1[:, m, :], zp)
                    zp2t = psrt.tile([128, TG, HID], F32, tag="rt",
                                     name="zp2")
                    zp2 = zp2t[:].rearrange("p a b -> p (a b)")
                    hh = HID // 2
                    for c in range(2):
                        last = (c == 1) and not flags["bc2"]
                        nc.tensor.matmul(
                            zp2[:, :512],
                            lhsT=Wc2b[:, c * hh : (c + 1) * hh],
                            rhs=z1[:, c, :], start=(c == 0), stop=last,
                        )
                    if flags["bc2"]:
                        nc.tensor.matmul(
                            zp2[:, :512], lhsT=consts["bc2"][:1, :],
                            rhs=ones[:1, :], start=False, stop=True,
                        )
                    z2 = ep.tile([128, 512], BF16, tag="z2")
                    elu_fused(z2[:], zp2)
                    zp3 = psz3.tile([NCLS, 512], F32, tag="zp3")
                    nc.tensor.matmul(
                        zp3[:], lhsT=Wc3b[: HID // 2, :], rhs=z2[:],
                        start=True, stop=True,
                    )
                    ot = wp.tile([NCLS, 512], F32, tag="ot")
                    nc.scalar.activation(
                        ot[:], zp3[:], AF.Identity, bias=bc3c[:, 0:1]
                    )
                    nc.sync.dma_start(out_t[:, b * 512 : (b + 1) * 512], ot[:])

    nc.compile()
    return nc


# ---------------------------------------------------------------- entry point

_CACHE = {}


def run(inputs, cfg, **kw):
    in_maps, meta = prepare_host(inputs, cfg)
    fl = tuple(sorted(meta["flags"].items()))
    key = (cfg.n, cfg.e, cfg.wpc, cfg.T, fl)
    if key not in _CACHE:
        _CACHE[key] = build_nc(cfg, meta["flags"])
    nc = _CACHE[key]
    res = run_bass_kernel_spmd(nc, in_maps, list(range(NCORES)), **kw)
    e = cfg.e
    outs = []
    for c in range(NCORES):
        e0 = min(c * cfg.ec, e)
        e1 = min(e0 + cfg.ec, e)
        outs.append(res.results[c]["out_t"].T[: e1 - e0])
    out = np.concatenate(outs, axis=0).astype(np.float32)
    return out, res


def kernel(**inputs) -> np.ndarray:
    n = inputs["x"].shape[0]
    e = inputs["edge_index"].shape[1]
    # wpc chosen so slots >= n; T from worst-case window load (verified in prep)
    wpc = -(-n // (NCORES * 128))
    cfg = Cfg(n, e, wpc=wpc, T=9, in_dim=inputs["x"].shape[1])
    # bump T if balancing can't fit (prepare_host asserts otherwise)
    while True:
        try:
            out, _ = run(inputs, cfg)
            return out
        except AssertionError as ex:
            if "window overflow" in str(ex) and cfg.T < 16:
                cfg = Cfg(n, e, wpc=wpc, T=cfg.T + 1, in_dim=inputs["x"].shape[1])
                continue
            raise


# revision 33
# speedup vs baseline: 1.0833x; 1.0833x over previous
"""Trainium2 Bass kernel for nn_ScoreGraphReconstructor (3-layer GATv2 + edge MLP).

Sharding: nodes are permuted into 8*WPC windows of 128 slots, balanced by
in-degree. Each core owns WPC windows (contiguous slot range) and all edges
whose *target* lands in its windows, so the segment softmax/scatter is fully
core-local.

v2 design (bf16 data path):
- All matmuls bf16 (fp8 one-hot stationaries), PSUM fp32 accumulate.
- Per layer: local node matmuls xl/xr (bf16) -> AllGather xl (bf16, HBM) ->
  edge phase per window: dma_gather of source transforms (bf16, 512B/edge),
  xr broadcast to edge slots via precomputed one-hot matmul (st2) instead of
  a second gather, leaky-relu on the scalar engine (Prelu), att-weighted
  group-reduce for logits on vector, exp on scalar, one-hot scatter matmul
  (st) accumulating [weighted features | softmax denominator] in PSUM.
- Edge classifier: transposed bf16 gathers from AllGathered h3, all-bf16
  matmuls, ELU via 2 scalar passes + 1 fused vector op, biases folded into
  scalar-engine activations or K=1 bf16 matmuls.
"""

import sys

for _p in ("/opt/trn_rl_repo",):
    if _p not in sys.path:
        sys.path.insert(0, _p)

import numpy as np
import ml_dtypes

import concourse.bass as bass
import concourse.bacc as bacc
import concourse.mybir as mybir
import concourse.tile as tile
from concourse.bass_utils import run_bass_kernel_spmd

F32 = mybir.dt.float32
BF16 = mybir.dt.bfloat16
FP8 = mybir.dt.float8e4
I16 = mybir.dt.int16

NPBF16 = ml_dtypes.bfloat16
NPFP8 = ml_dtypes.float8_e4m3

NCORES = 8
H, C = 4, 64
HID = H * C  # 256
NCLS = 5


class Cfg:
    def __init__(self, n_nodes, n_edges, wpc, T, in_dim=256):
        self.n = n_nodes
        self.e = n_edges
        self.wpc = wpc                    # windows per core
        self.T = T                        # edge tiles (of 128) per window
        self.L = wpc * 128                # local slots per core
        self.S = NCORES * self.L          # total slots
        self.nwin = NCORES * self.wpc
        self.ew = T * 128                 # edge slots per window
        self.in_dim = in_dim
        self.ec = -(-n_edges // NCORES)   # classifier edges per core (unpadded)
        self.ecp = -(-self.ec // 512) * 512  # padded to 512


# ---------------------------------------------------------------- host prep


def _balance_windows(deg, nwin):
    """Assign node n (with weight deg[n]) to one of nwin windows, each holding
    exactly 128 nodes (rest dummy), minimizing max window load. Greedy LPT."""
    import heapq

    n = len(deg)
    order = np.argsort(-deg, kind="stable")
    heap = [(0, w) for w in range(nwin)]
    heapq.heapify(heap)
    count = np.zeros(nwin, np.int64)
    slot_of = np.empty(n, np.int64)
    pos = np.zeros(nwin, np.int64)
    for node in order:
        while True:
            load, w = heapq.heappop(heap)
            if count[w] < 128:
                break
        slot_of[node] = w * 128 + pos[w]
        pos[w] += 1
        count[w] += 1
        if count[w] < 128:
            heapq.heappush(heap, (load + int(deg[node]), w))
    return slot_of


def _wrap_idx16(idx, rows=128):
    """dma_gather index layout: index i lives at [i % 16, i // 16] of a
    (rows, len/16) int16 SBUF tile; rows 16..127 padded with zeros."""
    n = len(idx)
    assert n % 16 == 0
    blk = np.asarray(idx, np.int16).reshape(n // 16, 16).T
    return np.tile(blk, (rows // 16, 1))


def prepare_host(inputs, cfg):
    """Build per-core input maps + metadata. inputs: dict from setup_inputs."""
    cn = {k: np.asarray(v) for k, v in inputs.items()}
    x = cn["x"].astype(np.float32)
    ei = cn["edge_index"].astype(np.int64)
    row, col = ei[0], ei[1]
    n, e = cfg.n, cfg.e
    L, S, wpc, T = cfg.L, cfg.S, cfg.wpc, cfg.T

    deg = np.bincount(col, minlength=n) + 1  # +1 self loop
    slot_of = _balance_windows(deg, cfg.nwin)

    # --- edge lists (conv graph: original edges + self loops on ALL slots)
    all_slots = np.arange(S, dtype=np.int64)
    src_sl = np.concatenate([slot_of[row], all_slots])
    dst_sl = np.concatenate([slot_of[col], all_slots])
    win = dst_sl // 128
    ordr = np.argsort(win, kind="stable")
    src_sl, dst_sl, win = src_sl[ordr], dst_sl[ordr], win[ordr]
    starts = np.searchsorted(win, np.arange(cfg.nwin))
    ends = np.searchsorted(win, np.arange(cfg.nwin), side="right")
    maxcnt = int((ends - starts).max())
    assert maxcnt <= cfg.ew, f"window overflow: {maxcnt} > {cfg.ew}; raise T"

    # per-core tensors: gather indices + one-hot scatter/broadcast matrices
    per_core = []
    for c in range(NCORES):
        esrc = np.zeros((128, wpc, cfg.ew // 16), np.int16)
        st = np.zeros((128, wpc, T, 128), NPFP8)   # [edge_part, dst] scatter
        st2 = np.zeros((128, wpc, T, 128), NPFP8)  # [dst_part, edge] broadcast
        for wi in range(wpc):
            w = c * wpc + wi
            s0, s1 = starts[w], ends[w]
            cnt = s1 - s0
            srcw = np.zeros(cfg.ew, np.int64)
            dofw = np.full(cfg.ew, 128, np.int64)
            srcw[:cnt] = src_sl[s0:s1]
            dofw[:cnt] = dst_sl[s0:s1] - w * 128
            esrc[:, wi, :] = _wrap_idx16(srcw)
            # edge ordinal i -> partition i%128, tile i//128 (gather layout)
            dof2 = dofw.reshape(T, 128)  # [t, p]
            for t in range(T):
                valid = dof2[t] < 128
                p_idx = np.nonzero(valid)[0]
                d_idx = dof2[t][valid]
                st[p_idx, wi, t, d_idx] = 1.0
                st2[d_idx, wi, t, p_idx] = 1.0
        per_core.append({
            "esrc": esrc,
            "st": st.reshape(128, wpc * T * 128),
            "st2": st2.reshape(128, wpc * T * 128),
        })

    # --- node features, permuted + transposed (bf16)
    xp = np.zeros((S, cfg.in_dim), np.float32)
    xp[slot_of] = x
    nin = cfg.in_dim // 128

    # --- weights (bf16) with ELU(-1) folds
    wmaps = {}
    flags = {}
    for l in (1, 2, 3):
        Wl = cn[f"W{l}l"].astype(np.float32)
        Wr = cn[f"W{l}r"].astype(np.float32)
        bl = cn[f"b{l}l"].astype(np.float32).copy()
        br = cn[f"b{l}r"].astype(np.float32).copy()
        att = cn[f"att{l}"].astype(np.float32)
        wmaps[f"Wl{l}"] = Wl.astype(NPBF16)
        wmaps[f"Wr{l}"] = Wr.astype(NPBF16)
        wmaps[f"bl{l}"] = bl.reshape(1, HID).astype(NPBF16)
        wmaps[f"br{l}"] = br.reshape(1, HID).astype(NPBF16)
        flags[f"bl{l}"] = bool(np.any(bl != 0))
        flags[f"br{l}"] = bool(np.any(br != 0))
        wmaps[f"attb{l}"] = np.tile(att.reshape(1, HID), (128, 1)).astype(NPBF16)
        gb = cn[f"bias{l}"].astype(np.float32)
        if l < 3:
            wmaps[f"biasb{l}"] = np.tile(gb.reshape(1, HID), (128, 1)).astype(NPBF16)
            flags[f"biasb{l}"] = bool(np.any(gb != 0))
        else:
            bias3 = gb  # folded into classifier bc1 below

    Wc1 = cn["Wc1"].astype(np.float32)
    Wc2 = cn["Wc2"].astype(np.float32)
    Wc3 = cn["Wc3"].astype(np.float32)
    bc1 = cn["bc1"].astype(np.float32) + np.concatenate([bias3, bias3]) @ Wc1
    bc2 = cn["bc2"].astype(np.float32)
    bc3 = cn["bc3"].astype(np.float32)
    wmaps["Wc1"] = Wc1.astype(NPBF16)
    wmaps["Wc2"] = Wc2.astype(NPBF16)
    wmaps["Wc3"] = Wc3.astype(NPBF16)
    wmaps["bc1"] = bc1.reshape(1, HID).astype(NPBF16)
    flags["bc1"] = bool(np.any(bc1 != 0))
    wmaps["bc2"] = bc2.reshape(1, HID // 2).astype(NPBF16)
    flags["bc2"] = bool(np.any(bc2 != 0))
    wmaps["bc3c"] = bc3.reshape(NCLS, 1).astype(np.float32)
    wmaps["ones512"] = np.ones((1, 512), NPBF16)
    wmaps["ident"] = np.eye(128, dtype=NPBF16)
    wmaps["ident8"] = np.eye(128, dtype=NPFP8)

    # --- classifier edge shards (original edge order)
    in_maps = []
    for c in range(NCORES):
        m = dict(wmaps)
        m.update(per_core[c])
        xc = xp[c * L : (c + 1) * L].T.copy()  # (in_dim, L)
        m["xfm"] = xc.reshape(nin, 128, L).astype(NPBF16)
        e0 = min(c * cfg.ec, e)
        e1 = min(e0 + cfg.ec, e)
        rowp = np.zeros(cfg.ecp, np.int64)
        colp = np.zeros(cfg.ecp, np.int64)
        rowp[: e1 - e0] = slot_of[row[e0:e1]]
        colp[: e1 - e0] = slot_of[col[e0:e1]]
        m["crow"] = _wrap_idx16(rowp)
        m["ccol"] = _wrap_idx16(colp)
        in_maps.append(m)

    meta = {"slot_of": slot_of, "cfg": cfg, "flags": flags}
    return in_maps, meta


# ---------------------------------------------------------------- device build


def build_nc(cfg, flags, debug=False):
    nc = bacc.Bacc(
        "TRN2",
        target_bir_lowering=False,
        debug=debug,
        num_devices=NCORES,
        num_swdge_queues=1,
    )
    L, S, wpc, T, ew = cfg.L, cfg.S, cfg.wpc, cfg.T, cfg.ew
    nin = cfg.in_dim // 128
    ecp = cfg.ecp
    AF = mybir.ActivationFunctionType
    OP = mybir.AluOpType

    # ---- I/O declarations
    P = {}

    def pin(name, shape, dtype=BF16):
        P[name] = nc.declare_dram_parameter(name, list(shape), dtype, isOutput=False)

    pin("xfm", (nin, 128, L))
    pin("esrc", (128, wpc, ew // 16), I16)
    pin("st", (128, wpc * T * 128), FP8)
    pin("st2", (128, wpc * T * 128), FP8)
    for l in (1, 2, 3):
        pin(f"Wl{l}", (cfg.in_dim if l == 1 else HID, HID))
        pin(f"Wr{l}", (cfg.in_dim if l == 1 else HID, HID))
        if flags[f"bl{l}"]:
            pin(f"bl{l}", (1, HID))
        if flags[f"br{l}"]:
            pin(f"br{l}", (1, HID))
        pin(f"attb{l}", (128, HID))
        if l < 3 and flags[f"biasb{l}"]:
            pin(f"biasb{l}", (128, HID))
    pin("Wc1", (2 * HID, HID))
    pin("Wc2", (HID, HID // 2))
    pin("Wc3", (HID // 2, NCLS))
    if flags["bc1"]:
        pin("bc1", (1, HID))
    if flags["bc2"]:
        pin("bc2", (1, HID // 2))
    pin("bc3c", (NCLS, 1), F32)
    pin("ones512", (1, 512))
    pin("ident", (128, 128))
    pin("ident8", (128, 128), FP8)
    pin("crow", (128, ecp // 16), I16)
    pin("ccol", (128, ecp // 16), I16)
    out_t = nc.declare_dram_parameter("out_t", [NCLS, ecp], F32, isOutput=True)

    rg = [list(range(NCORES))]
    # rt psum processed in groups of TG tiles (PSUM is 8 banks of 2KB)
    TG = 2

    with tile.TileContext(nc) as tc:
        with (
            tc.tile_pool(name="const", bufs=1) as cp,
            tc.tile_pool(name="dram", bufs=1, space="DRAM") as dp,
            tc.tile_pool(name="work", bufs=2) as wp,
            tc.tile_pool(name="epbig", bufs=2) as ep,
            tc.tile_pool(name="psrt", bufs=4, space="PSUM") as psrt,
            tc.tile_pool(name="psout", bufs=1, space="PSUM") as psout,
            tc.tile_pool(name="pstp", bufs=1, space="PSUM") as pstp,
            tc.tile_pool(name="psz3", bufs=1, space="PSUM") as psz3,
        ):
            # ---------- constants into SBUF
            # tensors with >128 rows are stored as column-chunks:
            # rows [c*128:(c+1)*128] live at columns [c*s1:(c+1)*s1]
            def load_const(name, dtype=BF16):
                src = P[name]
                shp = list(src.shape)
                if len(shp) == 2 and shp[0] > 128:
                    s0, s1 = shp
                    nchunk = s0 // 128
                    t = cp.tile([128, nchunk * s1], dtype, tag=name, name=name + "_sb")
                    for c in range(nchunk):
                        nc.sync.dma_start(
                            t[:, c * s1 : (c + 1) * s1], src[c * 128 : (c + 1) * 128, :]
                        )
                else:
                    t = cp.tile(shp, dtype, tag=name, name=name + "_sb")
                    nc.sync.dma_start(t[:], src[:])
                return t

            def wchunk(t, name, c):
                """chunk c (rows c*128..) of a >128-row const loaded by load_const."""
                s1 = P[name].shape[1]
                return t[:, c * s1 : (c + 1) * s1]

            xfm = [
                cp.tile([128, L], BF16, tag=f"xfm{c}", name=f"xfm{c}")
                for c in range(nin)
            ]
            for c in range(nin):
                nc.sync.dma_start(xfm[c][:], P["xfm"][c])
            esrc_sb = load_const("esrc", dtype=I16)
            st_sb = load_const("st", dtype=FP8)
            st2_sb = load_const("st2", dtype=FP8)
            stv = st_sb[:].rearrange("p (w t d) -> p w t d", t=T, d=128)
            st2v = st2_sb[:].rearrange("p (w t d) -> p w t d", t=T, d=128)
            consts = {}
            for l in (1, 2, 3):
                for nm in (f"Wl{l}", f"Wr{l}", f"bl{l}", f"br{l}", f"attb{l}",
                           f"biasb{l}"):
                    if nm in P:
                        consts[nm] = load_const(nm)
            for nm in ("Wc1", "Wc2", "Wc3", "bc1", "bc2", "ones512"):
                if nm in P:
                    consts[nm] = load_const(nm)
            consts["bc3c"] = load_const("bc3c", dtype=F32)
            crow_sb = load_const("crow", dtype=I16)
            ccol_sb = load_const("ccol", dtype=I16)
            ident = load_const("ident")
            ident8 = load_const("ident8", dtype=FP8)

            hbuf = cp.tile([128, wpc, HID], BF16, tag="hbuf")
            xr_sb = cp.tile([128, wpc, HID], BF16, tag="xr_sb")

            # DRAM scratch
            xl_in = {
                l: dp.tile([L, HID], BF16, tag=f"xl_in{l}", name=f"xl_in{l}")
                for l in (1, 2, 3)
            }
            xl_full = {
                l: dp.tile([S, HID], BF16, tag=f"xl_full{l}", name=f"xl_full{l}",
                            addr_space="Shared")
                for l in (1, 2, 3)
            }
            h3_in = dp.tile([L, HID], BF16, tag="h3_in")
            h3_full = dp.tile([S, HID], BF16, tag="h3_full", addr_space="Shared")

            ones = consts["ones512"]

            # ================= layers
            for l in (1, 2, 3):
                kin = cfg.in_dim if l == 1 else HID
                nk = kin // 128
                # ---- transposes of hbuf into xfm for l >= 2
                if l >= 2:
                    for w in range(wpc):
                        ws = slice(w * 128, (w + 1) * 128)
                        for c in range(nk):
                            tps = pstp.tile([128, 128], BF16, tag="tps")
                            nc.tensor.transpose(
                                tps[:], hbuf[:, w, c * 128 : (c + 1) * 128], ident[:]
                            )
                            nc.scalar.activation(xfm[c][:, ws], tps[:], AF.Copy)
                # ---- node matmuls: xl (to DRAM for AllGather), xr (SBUF)
                for w in range(wpc):
                    for side, Wn, bn, to_dram in (
                        ("l", f"Wl{l}", f"bl{l}", True),
                        ("r", f"Wr{l}", f"br{l}", False),
                    ):
                        ws = slice(w * 128, (w + 1) * 128)
                        ps = psout.tile([128, HID + H], F32,
                                        tag=f"ops{0 if to_dram else 1}",
                                        name="ps_node")
                        pso = ps[:, :HID]
                        for c in range(nk):
                            nc.tensor.matmul(
                                pso,
                                lhsT=xfm[c][:, ws],
                                rhs=wchunk(consts[Wn], Wn, c),
                                start=(c == 0),
                                stop=(c == nk - 1 and bn not in consts),
                            )
                        if bn in consts:
                            # bias via K=1 outer product: ones(1,128)^T @ b(1,256)
                            nc.tensor.matmul(
                                pso,
                                lhsT=ones[:1, 0:128],
                                rhs=consts[bn][:1, :],
                                start=False,
                                stop=True,
                            )
                        if to_dram:
                            xo = wp.tile([128, HID], BF16, tag="xo")
                            nc.scalar.activation(xo[:], pso, AF.Copy)
                            nc.sync.dma_start(xl_in[l][ws, :], xo[:])
                        else:
                            nc.scalar.activation(xr_sb[:, w, :], pso, AF.Copy)

                # ---- AllGather xl
                nc.gpsimd.collective_compute(
                    "AllGather",
                    OP.bypass,
                    replica_groups=rg,
                    ins=[xl_in[l][:].opt()],
                    outs=[xl_full[l][:].opt()],
                )

                # ---- edge phase: windows processed in pairs (fewer, larger
                # gathers and vector ops; deeper cross-engine pipelining)
                attb = consts[f"attb{l}"]
                for w0 in range(0, wpc, 2):
                    nw = min(2, wpc - w0)
                    nt = nw * T
                    gt = ep.tile([128, 2 * T, HID], BF16, tag="gt")
                    gv = gt[:, :nt, :]
                    nc.gpsimd.dma_gather(
                        out_ap=gv,
                        in_ap=xl_full[l][:],
                        idxs_ap=esrc_sb[:, w0 : w0 + nw, :],
                        num_idxs=nw * ew,
                        num_idxs_reg=nw * ew,
                        elem_size=HID,
                        single_packet=False,
                        queue_num=0,
                    )
                    # A = st2 @ xr + ident @ gt, all accumulated on the PE;
                    # leaky via Prelu on scalar straight from PSUM.
                    am = ep.tile([128, 2 * T, HID], BF16, tag="am")
                    av = am[:, :nt, :]
                    for g0 in range(0, nt, TG):
                        gn = min(TG, nt - g0)
                        rt = psrt.tile([128, TG, HID], F32, tag="rt")
                        for t in range(gn):
                            gt_t = g0 + t
                            nc.tensor.matmul(
                                rt[:, t, :],
                                lhsT=st2v[:, w0 + gt_t // T, gt_t % T, :],
                                rhs=xr_sb[:, w0 + gt_t // T, :],
                                start=True,
                                stop=True,
                            )
                        nc.vector.tensor_add(
                            am[:, g0 : g0 + gn, :], gt[:, g0 : g0 + gn, :],
                            rt[:, :gn, :],
                        )
                    # E = leaky_relu(A, 0.2) on scalar; F = E*att
                    nc.scalar.activation(av, av, AF.Prelu, alpha=0.2)
                    nc.vector.tensor_tensor(
                        out=av,
                        in0=av,
                        in1=attb[:].unsqueeze(1).to_broadcast([128, nt, HID]),
                        op=OP.mult,
                    )
                    lg = ep.tile([128, 2 * T, H], F32, tag="lg")
                    nc.vector.tensor_reduce(
                        out=lg[:, :nt, :],
                        in_=av.rearrange("p t (h c) -> p t h c", c=C),
                        axis=mybir.AxisListType.X,
                        op=OP.add,
                    )
                    wx = ep.tile([128, 2 * T, HID + H], BF16, tag="wx")
                    nc.scalar.activation(
                        wx[:, :nt, HID : HID + H], lg[:, :nt, :], AF.Exp
                    )
                    nc.vector.tensor_tensor(
                        out=wx[:, :nt, 0:HID].rearrange("p t (h c) -> p t h c", c=C),
                        in0=gv.rearrange("p t (h c) -> p t h c", c=C),
                        in1=wx[:, :nt, HID : HID + H]
                        .unsqueeze(3)
                        .to_broadcast([128, nt, H, C]),
                        op=OP.mult,
                    )
                    opsl = [
                        psout.tile([128, HID + H], F32, tag=f"ops{wi}",
                                   name=f"ops{wi}")
                        for wi in range(nw)
                    ]
                    for wi in range(nw):
                        for t in range(T):
                            nc.tensor.matmul(
                                opsl[wi][:],
                                lhsT=stv[:, w0 + wi, t, :],
                                rhs=wx[:, wi * T + t, :],
                                start=(t == 0),
                                stop=(t == T - 1),
                            )
                    rc = ep.tile([128, 2, H], F32, tag="rc")
                    for wi in range(nw):
                        nc.vector.reciprocal(
                            rc[:, wi, :], opsl[wi][:, HID : HID + H]
                        )
                        nc.vector.tensor_tensor(
                            out=hbuf[:, w0 + wi, :].rearrange("p (h c) -> p h c", c=C),
                            in0=opsl[wi][:, 0:HID].rearrange("p (h c) -> p h c", c=C),
                            in1=rc[:, wi, :].unsqueeze(2).to_broadcast([128, H, C]),
                            op=OP.mult,
                        )

                if l < 3 and flags[f"biasb{l}"]:
                    nc.vector.tensor_tensor(
                        out=hbuf[:],
                        in0=hbuf[:],
                        in1=consts[f"biasb{l}"][:]
                        .unsqueeze(1)
                        .to_broadcast([128, wpc, HID]),
                        op=OP.add,
                    )

                if l <= 2:
                    # ELU = exp(min(h,0)) - 1 + max(h,0). Storing elu+1 in
                    # bf16 and folding -1 into the next bias loses absolute
                    # precision for small h; subtract the 1 here instead.
                    eg = 5
                    for g in range(0, wpc, eg):
                        ge = min(g + eg, wpc)
                        sl = hbuf[:, g:ge, :]
                        # te holds exp(min(h,0)) ~ 1 for small h; keep fp32 so
                        # the -1 subtraction doesn't amplify rounding error
                        te = wp.tile([128, eg, HID], F32, tag="te")
                        tp = wp.tile([128, eg, HID], BF16, tag="tp")
                        tv = te[:, : ge - g, :]
                        pv = tp[:, : ge - g, :]
                        nc.scalar.activation(tv, sl, AF.Relu, scale=-1.0)
                        nc.scalar.activation(tv, tv, AF.Exp, scale=-1.0)
                        nc.scalar.activation(pv, sl, AF.Relu)
                        nc.vector.scalar_tensor_tensor(
                            out=sl, in0=tv, scalar=-1.0, in1=pv,
                            op0=OP.add, op1=OP.add,
                        )
                else:
                    for w in range(wpc):
                        nc.sync.dma_start(
                            h3_in[w * 128 : (w + 1) * 128, :], hbuf[:, w, :]
                        )

            # ---- AllGather h3 (bf16)
            nc.gpsimd.collective_compute(
                "AllGather",
                OP.bypass,
                replica_groups=rg,
                ins=[h3_in[:].opt()],
                outs=[h3_full[:].opt()],
            )

            # ================= classifier
            Wc1b = consts["Wc1"]
            Wc2b = consts["Wc2"]
            Wc3b = consts["Wc3"]
            bc3c = consts["bc3c"]

            def elu_fused(dst, zp):
                """dst(bf16) = elu(zp) = exp(min(zp,0)) - 1 + max(zp,0).
                The -1 must be applied before any bf16 rounding of the
                exp term (~1 for small z), so te stays fp32 and the fused
                vector op does (te - 1) + relu(z). relu(z) on vector to
                offload the classifier-phase-saturated scalar engine."""
                te = ep.tile([128, 512], F32, tag="clf_te")
                tp = ep.tile([128, 512], BF16, tag="clf_tp")
                nc.scalar.activation(te[:], zp[:], AF.Relu, scale=-1.0)
                nc.scalar.activation(te[:], te[:], AF.Exp, scale=-1.0)
                nc.vector.tensor_scalar(
                    out=tp[:], in0=zp[:], scalar1=0.0, scalar2=None, op0=OP.max
                )
                nc.vector.scalar_tensor_tensor(
                    out=dst, in0=te[:], scalar=-1.0, in1=tp[:],
                    op0=OP.add, op1=OP.add,
                )

            # blocks of 512 edges, gathered in pairs (1024 idx per gather)
            for pb in range(ecp // 1024):
                ps_ = slice(pb * 64, (pb + 1) * 64)
                ut = ep.tile([128, HID // 128, 1024], BF16, tag="ut")
                nc.gpsimd.dma_gather(
                    out_ap=ut[:], in_ap=h3_full[:], idxs_ap=crow_sb[:, ps_],
                    num_idxs=1024, num_idxs_reg=1024, elem_size=HID,
                    transpose=True, single_packet=False, queue_num=0,
                )
                vt = ep.tile([128, HID // 128, 1024], BF16, tag="vt")
                nc.gpsimd.dma_gather(
                    out_ap=vt[:], in_ap=h3_full[:], idxs_ap=ccol_sb[:, ps_],
                    num_idxs=1024, num_idxs_reg=1024, elem_size=HID,
                    transpose=True, single_packet=False, queue_num=0,
                )
                for sb in range(2):
                    b = 2 * pb + sb
                    ss = slice(sb * 512, (sb + 1) * 512)
                    z1 = ep.tile([128, 2, 512], BF16, tag="z1")
                    for m in range(2):
                        ms = slice(m * 128, (m + 1) * 128)
                        zpt = psrt.tile([128, TG, HID], F32, tag="rt",
                                        name="zp1")
                        zp = zpt[:].rearrange("p a b -> p (a b)")
                        for c in range(2):
                            nc.tensor.matmul(
                                zp[:],
                                lhsT=Wc1b[
                                    :, c * HID + m * 128 : c * HID + (m + 1) * 128
                                ],
                                rhs=ut[:, c, ss], start=(c == 0), stop=False,
                            )
                        for c in range(2):
                            cc = 2 + c
                            last = (c == 1) and not flags["bc1"]
                            nc.tensor.matmul(
                                zp[:],
                                lhsT=Wc1b[
                                    :, cc * HID + m * 128 : cc * HID + (m + 1) * 128
                                ],
                                rhs=vt[:, c, ss], start=False, stop=last,
                            )
                        if flags["bc1"]:
                            nc.tensor.matmul(
                                zp[:], lhsT=consts["bc1"][:1, ms], rhs=ones[:1, :],
                                start=False, stop=True,
                            )
                        elu_fused(z1[:, m, :], zp)
                    zp2t = psrt.tile([128, TG, HID], F32, tag="rt",
                                     name="zp2")
                    zp2 = zp2t[:].rearrange("p a b -> p (a b)")
                    hh = HID // 2
                    for c in range(2):
                        last = (c == 1) and not flags["bc2"]
                        nc.tensor.matmul(
                            zp2[:, :512],
                            lhsT=Wc2b[:, c * hh : (c + 1) * hh],
                            rhs=z1[:, c, :], start=(c == 0), stop=last,
                        )
                    if flags["bc2"]:
                        nc.tensor.matmul(
                            zp2[:, :512], lhsT=consts["bc2"][:1, :],
                            rhs=ones[:1, :], start=False, stop=True,
                        )
                    z2 = ep.tile([128, 512], BF16, tag="z2")
                    elu_fused(z2[:], zp2)
                    zp3 = psz3.tile([NCLS, 512], F32, tag="zp3")
                    nc.tensor.matmul(
                        zp3[:], lhsT=Wc3b[: HID // 2, :], rhs=z2[:],
                        start=True, stop=True,
                    )
                    ot = wp.tile([NCLS, 512], F32, tag="ot")
                    nc.scalar.activation(
                        ot[:], zp3[:], AF.Identity, bias=bc3c[:, 0:1]
                    )
                    nc.sync.dma_start(out_t[:, b * 512 : (b + 1) * 512], ot[:])

    nc.compile()
    return nc


# ---------------------------------------------------------------- entry point

_CACHE = {}


def run(inputs, cfg, **kw):
    in_maps, meta = prepare_host(inputs, cfg)
    fl = tuple(sorted(meta["flags"].items()))
    key = (cfg.n, cfg.e, cfg.wpc, cfg.T, fl)
    if key not in _CACHE:
        _CACHE[key] = build_nc(cfg, meta["flags"])
    nc = _CACHE[key]
    res = run_bass_kernel_spmd(nc, in_maps, list(range(NCORES)), **kw)
    e = cfg.e
    outs = []
    for c in range(NCORES):
        e0 = min(c * cfg.ec, e)
        e1 = min(e0 + cfg.ec, e)
        outs.append(res.results[c]["out_t"].T[: e1 - e0])
    out = np.concatenate(outs, axis=0).astype(np.float32)
    return out, res


def kernel(**inputs) -> np.ndarray:
    n = inputs["x"].shape[0]
    e = inputs["edge_index"].shape[1]
    # wpc chosen so slots >= n; T from worst-case window load (verified in prep)
    wpc = -(-n // (NCORES * 128))
    cfg = Cfg(n, e, wpc=wpc, T=9, in_dim=inputs["x"].shape[1])
    # bump T if balancing can't fit (prepare_host asserts otherwise)
    while True:
        try:
            out, _ = run(inputs, cfg)
            return out
        except AssertionError as ex:
            if "window overflow" in str(ex) and cfg.T < 16:
                cfg = Cfg(n, e, wpc=wpc, T=cfg.T + 1, in_dim=inputs["x"].shape[1])
                continue
            raise
